# revision 19
# baseline (speedup 1.0000x reference)
"""PointLink loss kernel for 8 trn2 NeuronCores (data-parallel over batch).

loss[img, d] = sum_{cells, slots} W(cell,ch) * (o - T(cell,ch))^2
             = sum W*o^2  - 2*sum W*T*o  + sum W*T^2
A-pass: ACT square (fp32 -> bf16) + fused mult-reduce (STT) vs W tile
B-pass: DVE slot-pair sum (fp32 -> bf16) + fused mult-reduce vs WT tile
        (WT = -2*W*T baked on device from per-cell indices)
C3: target-only constant, computed on host.

Device layout: partition p = (img_local 32, quarter 4); per (batch, d, chunk)
free dim = (cells 25/24, slot 4, ch 51) contiguous in DRAM.
bf16 tiles are padded so every STT operand is 4B-aligned with step-1
innermost dim (DVE 2x_1P mode).
"""

import sys

sys.path.insert(0, "/opt/trn_rl_repo")

import numpy as np

import concourse.bass as bass
import concourse.bacc as bacc
import concourse.tile as tile
from concourse import mybir
from concourse.bass_utils import run_bass_kernel_spmd

GRID = 14
B = 2
CLASSES = 20
BATCH = 512
NBOX = 32
NCELL = GRID * GRID          # 196
CH = 51
CHP = 52                     # padded channel stride (bf16 alignment)
SLOTS = 2 * B                # 4
SC = SLOTS * CH              # 204
PER_D = NCELL * SC           # 39984
PER_IMG = 4 * PER_D          # 159936
NCORES = 8
IMGS_PER_CORE = BATCH // NCORES   # 64
IMGS_PER_BATCH = 32               # images per device batch (x4 quarters = 128p)
NBATCH = IMGS_PER_CORE // IMGS_PER_BATCH  # 2
QCELLS = NCELL // 4               # 49 cells per quarter
CHUNKS = (25, 24)                 # cell sub-chunks within a quarter
NCHUNK = len(CHUNKS)
NAUX = 18                         # per-cell aux fields
NGRP = 7                          # W_c, W_z, WT_c, WT_z d0..d3
TW = NGRP * CHP                   # 364, T_all per-cell stride (bf16, aligned)
SQBLK = CHUNKS[0] * CH + CH       # 1326: per-slot block stride in sq tile (even)
SEG = CHUNKS[0] * CH + 1          # 1276: product segment stride (even)
NACC = 4 * NCHUNK                 # one acc column per (d, chunk)

F32 = mybir.dt.float32
BF16 = mybir.dt.bfloat16


def _reap(ap, newap, extra_offset=0):
    return bass.AP(tensor=ap.tensor, offset=ap.offset + extra_offset, ap=newap)


# ----------------------------------------------------------------------------
# device program (built once per process)
# ----------------------------------------------------------------------------
_PROG = None


def _build_program():
    from contextlib import ExitStack

    nc = bacc.Bacc("TRN2", target_bir_lowering=False, debug=False,
                   num_devices=NCORES)
    x = nc.declare_dram_parameter("x", [IMGS_PER_CORE * PER_IMG], F32,
                                  isOutput=False)
    aux = nc.declare_dram_parameter("aux", [NBATCH, 128, NAUX * QCELLS], F32,
                                    isOutput=False)
    wvec = nc.declare_dram_parameter("wvec", [TW], BF16, isOutput=False)
    accout = nc.declare_dram_parameter("acc", [NBATCH, 128, NACC], F32,
                                       isOutput=True)

    with tile.TileContext(nc) as tc, ExitStack() as ctx:
        singles = ctx.enter_context(tc.tile_pool(name="singles", bufs=1))
        opool = ctx.enter_context(tc.tile_pool(name="opool", bufs=2))
        sqpool = ctx.enter_context(tc.tile_pool(name="sqpool", bufs=2))
        ospool = ctx.enter_context(tc.tile_pool(name="ospool", bufs=2))
        tpool = ctx.enter_context(tc.tile_pool(name="tpool", bufs=2))
        apool = ctx.enter_context(tc.tile_pool(name="apool", bufs=2))
        accpool = ctx.enter_context(tc.tile_pool(name="accpool", bufs=2))
        scrpool = ctx.enter_context(tc.tile_pool(name="scrpool", bufs=2))

        # constants
        t_wvec = singles.tile([128, TW], BF16)
        nc.gpsimd.dma_start(out=t_wvec[:], in_=wvec[:].partition_broadcast(128))
        t_iota = singles.tile([128, QCELLS, CH], F32)
        nc.gpsimd.iota(t_iota[:], pattern=[[0, QCELLS], [1, CH]], base=0,
                       channel_multiplier=0,
                       allow_small_or_imprecise_dtypes=True)

        for b in range(NBATCH):
            # target tile, bf16: per cell NGRP groups of CHP (double buffered
            # so batch b+1's build overlaps batch b's tail)
            t_all = tpool.tile([128, QCELLS * TW], BF16, tag="t_all")
            # ---- load aux, build targets ----
            t_aux = apool.tile([128, NAUX, QCELLS], F32)
            nc.gpsimd.dma_start(
                out=t_aux[:],
                in_=aux[:].rearrange("b p f -> b p f")[b])

            def fld_b(i, k):
                """aux field i broadcast over k columns: [128, QCELLS, k]"""
                a = t_aux[:, i, :]
                return _reap(a, [a.ap[0], [1, QCELLS], [0, k]])

            P = t_all[:].ap[0]

            def tview(grp, c0, ncols):
                return _reap(t_all[:], [P, [TW, QCELLS], [1, ncols]],
                             extra_offset=grp * CHP + c0)

            def ioview(c0, ncols):
                a = t_iota[:]
                return _reap(a, [a.ap[0], [0, QCELLS], [1, ncols]],
                             extra_offset=c0)

            iseq = mybir.AluOpType.is_equal

            # raw W_c / W_z: mask replicated over 51 ch
            nc.gpsimd.tensor_copy(out=tview(0, 0, CH), in_=fld_b(0, CH))
            nc.gpsimd.tensor_copy(out=tview(1, 0, CH), in_=fld_b(1, CH))
            # WT_c raw: conf=mask_c, cls+coord via is_equal (coord cols get 0),
            # links via is_equal
            nc.gpsimd.tensor_copy(out=tview(2, 0, 1), in_=fld_b(0, 1))
            nc.vector.tensor_tensor(out=tview(2, 1, 22), in0=fld_b(2, 22),
                                    in1=ioview(1, 22), op=iseq)
            nc.vector.tensor_tensor(out=tview(2, 23, 14), in0=fld_b(3, 14),
                                    in1=ioview(23, 14), op=iseq)
            nc.vector.tensor_tensor(out=tview(2, 37, 14), in0=fld_b(4, 14),
                                    in1=ioview(37, 14), op=iseq)
            # WT_z d0 raw
            nc.gpsimd.tensor_copy(out=tview(3, 0, 1), in_=fld_b(1, 1))
            nc.vector.tensor_tensor(out=tview(3, 1, 22), in0=fld_b(5, 22),
                                    in1=ioview(1, 22), op=iseq)
            # replicate z-common into d1..d3 before overwriting links
            for d in range(1, 4):
                nc.gpsimd.tensor_copy(out=tview(3 + d, 0, 23),
                                      in_=tview(3, 0, 23))
            # links per d
            for d in range(4):
                nc.vector.tensor_tensor(out=tview(3 + d, 23, 14),
                                        in0=fld_b(6 + d, 14),
                                        in1=ioview(23, 14), op=iseq)
                nc.vector.tensor_tensor(out=tview(3 + d, 37, 14),
                                        in0=fld_b(10 + d, 14),
                                        in1=ioview(37, 14), op=iseq)
            # scale all groups by per-ch weights (in place, bf16 2x on DVE)
            flat = _reap(t_all[:], [P, [TW, QCELLS], [1, TW]])
            wv_b = _reap(t_wvec[:], [t_wvec[:].ap[0], [0, QCELLS], [1, TW]])
            nc.vector.tensor_tensor(out=flat, in0=flat, in1=wv_b,
                                    op=mybir.AluOpType.mult)
            # coord targets (host pre-scaled: -mask*crd), after scale
            crd_c = _reap(t_aux[:], [t_aux[:].ap[0], [1, QCELLS], [QCELLS, 2]],
                          extra_offset=14 * QCELLS)
            nc.gpsimd.tensor_copy(out=tview(2, 21, 2), in_=crd_c)
            crd_z = _reap(t_aux[:], [t_aux[:].ap[0], [1, QCELLS], [QCELLS, 2]],
                          extra_offset=16 * QCELLS)
            for d in range(4):
                nc.gpsimd.tensor_copy(out=tview(3 + d, 21, 2), in_=crd_z)
            # conf weight of W_c / W_z is 1 for every cell
            nc.vector.memset(tview(0, 0, 1), 1.0)
            nc.vector.memset(tview(1, 0, 1), 1.0)

            # ---- accumulators ----
            t_acc = accpool.tile([128, NACC], F32)

            # ---- main streaming loop ----
            for d in range(4):
                c0 = 0
                for ci, ncl in enumerate(CHUNKS):
                    o_t = opool.tile([128, CHUNKS[0] * SC], F32, tag="o")
                    sq_t = sqpool.tile([128, SLOTS * SQBLK], BF16, tag="sq")
                    os_t = ospool.tile([128, CHUNKS[0] * 2 * CHP], BF16,
                                       tag="os")
                    pr_t = scrpool.tile([128, 4 * SEG], BF16, tag="prod")
                    nfree = ncl * SC
                    src = _reap(
                        x[:],
                        [[PER_IMG, IMGS_PER_BATCH], [QCELLS * SC, 4],
                         [1, nfree]],
                        extra_offset=(b * IMGS_PER_BATCH * PER_IMG
                                      + d * PER_D + c0 * SC))
                    nc.sync.dma_start(out=o_t[:, 0:nfree], in_=src)

                    op = o_t[:].ap[0]

                    def oview(slot):
                        """o slot view [P, (cell), (ch)]"""
                        return _reap(o_t[:], [op, [SC, ncl], [1, CH]],
                                     extra_offset=slot * CH)

                    def sqview(slot):
                        """sq slot view [P, (cell), (ch)] contiguous"""
                        return _reap(sq_t[:],
                                     [sq_t[:].ap[0], [CH, ncl], [1, CH]],
                                     extra_offset=slot * SQBLK)

                    def osview(grp):
                        return _reap(os_t[:],
                                     [os_t[:].ap[0], [2 * CHP, ncl], [1, CH]],
                                     extra_offset=grp * CHP)

                    def wview(grp):
                        return _reap(t_all[:], [P, [TW, ncl], [1, CH]],
                                     extra_offset=c0 * TW + grp * CHP)

                    def prview(seg):
                        return _reap(pr_t[:],
                                     [pr_t[:].ap[0], [CH, ncl], [1, CH]],
                                     extra_offset=seg * SEG)

                    # squares per slot (fp32 -> bf16)
                    for s in range(SLOTS):
                        nc.scalar.activation(
                            out=sqview(s), in_=oview(s),
                            func=mybir.ActivationFunctionType.Square)
                    add = mybir.AluOpType.add
                    mult = mybir.AluOpType.mult
                    # sq slot-pair sums in place (bf16, 2x) on DVE
                    nc.vector.tensor_tensor(out=sqview(0), in0=sqview(0),
                                            in1=sqview(1), op=add)
                    nc.vector.tensor_tensor(out=sqview(2), in0=sqview(2),
                                            in1=sqview(3), op=add)
                    # o slot-pair sums (fp32 -> bf16) on Pool
                    nc.gpsimd.tensor_tensor(out=osview(0), in0=oview(0),
                                            in1=oview(1), op=add)
                    nc.gpsimd.tensor_tensor(out=osview(1), in0=oview(2),
                                            in1=oview(3), op=add)

                    # weighted products into padded segments (bf16, 2x)
                    nc.vector.tensor_tensor(out=prview(0), in0=sqview(0),
                                            in1=wview(0), op=mult)
                    nc.vector.tensor_tensor(out=prview(1), in0=sqview(2),
                                            in1=wview(1), op=mult)
                    nc.vector.tensor_tensor(out=prview(2), in0=osview(0),
                                            in1=wview(2), op=mult)
                    nc.vector.tensor_tensor(out=prview(3), in0=osview(1),
                                            in1=wview(3 + d), op=mult)
                    # zero pads (keep inner dim even for the 4x reduce), then
                    # one reduce of all products into this (d, chunk) column
                    npad = 1 if (ncl * CH) % 2 else 2
                    pads = _reap(pr_t[:], [pr_t[:].ap[0], [SEG, 4], [1, npad]],
                                 extra_offset=ncl * CH)
                    nc.vector.memset(pads, 0.0)
                    nfl = ncl * CH + npad
                    flatpr = _reap(pr_t[:], [pr_t[:].ap[0], [SEG, 4], [1, nfl]])
                    col = d * NCHUNK + ci
                    nc.vector.tensor_scalar(
                        out=flatpr, in0=flatpr, scalar1=1.0, scalar2=0.0,
                        op0=mult, op1=add,
                        accum_out=t_acc[:, col:col + 1])
                    c0 += ncl

            nc.sync.dma_start(out=accout[:].rearrange("b p c -> b p c")[b],
                              in_=t_acc[:])

    nc.compile()
    return nc


# ----------------------------------------------------------------------------
# host-side target preparation
# ----------------------------------------------------------------------------
def _prepare(bboxes, labels, W):
    """Returns aux [BATCH, 4(quarter), NAUX, QCELLS] float32 and c3 [BATCH]."""
    g = bboxes.astype(np.float64) * (GRID / float(W))
    x0, y0, w, h = g[:, :, 0], g[:, :, 1], g[:, :, 2], g[:, :, 3]
    px = np.stack([x0, x0 + w, x0, x0 + w], axis=2)          # [B,N,4]
    py = np.stack([y0, y0, y0 + h, y0 + h], axis=2)
    px32 = px.astype(np.float32)
    py32 = py.astype(np.float32)
    pix = np.clip(np.floor(px32), 0, GRID - 1).astype(np.int64)
    piy = np.clip(np.floor(py32), 0, GRID - 1).astype(np.int64)
    pdx = (px32 - pix).astype(np.float32)
    pdy = (py32 - piy).astype(np.float32)
    cx = (x0 + w * 0.5).astype(np.float32)
    cy = (y0 + h * 0.5).astype(np.float32)
    cix = np.clip(np.floor(cx), 0, GRID - 1).astype(np.int64)
    ciy = np.clip(np.floor(cy), 0, GRID - 1).astype(np.int64)
    cdx = (cx - cix).astype(np.float32)
    cdy = (cy - ciy).astype(np.float32)

    nb = bboxes.shape[0]
    # corner first-match: flat scatter-min over (img, cell)
    corner_cell = (pix * GRID + piy).reshape(nb, 4 * NBOX)   # box-major
    which_c = np.full((nb, NCELL), 4 * NBOX, np.int64)
    imgix = np.repeat(np.arange(nb), 4 * NBOX)
    flat = imgix * NCELL + corner_cell.reshape(-1)
    wc_flat = which_c.reshape(-1)
    np.minimum.at(wc_flat, flat, np.tile(np.arange(4 * NBOX), nb))
    which_c = wc_flat.reshape(nb, NCELL)
    mask_c = which_c < 4 * NBOX
    wc = np.where(mask_c, which_c, 0)
    box_c = wc // 4

    center_cell = cix * GRID + ciy                            # [B,N]
    which_z = np.full((nb, NCELL), NBOX, np.int64)
    imgix2 = np.repeat(np.arange(nb), NBOX)
    flat2 = imgix2 * NCELL + center_cell.reshape(-1)
    wz_flat = which_z.reshape(-1)
    np.minimum.at(wz_flat, flat2, np.tile(np.arange(NBOX), nb))
    which_z = wz_flat.reshape(nb, NCELL)
    mask_z = which_z < NBOX
    wz = np.where(mask_z, which_z, 0)

    ii = np.arange(nb)[:, None]
    pdx_f = pdx.reshape(nb, -1)
    pdy_f = pdy.reshape(nb, -1)

    mc = mask_c.astype(np.float32)
    mzf = mask_z.astype(np.float32)
    aux = np.full((nb, NCELL, NAUX), -1.0, np.float32)
    aux[:, :, 0] = mc
    aux[:, :, 1] = mzf
    aux[:, :, 2] = np.where(mask_c, 1 + labels[ii, box_c], -1)
    aux[:, :, 3] = np.where(mask_c, 23 + cix[ii, box_c], -1)
    aux[:, :, 4] = np.where(mask_c, 37 + ciy[ii, box_c], -1)
    aux[:, :, 5] = np.where(mask_z, 1 + labels[ii, wz], -1)
    for d in range(4):
        aux[:, :, 6 + d] = np.where(mask_z, 23 + pix[ii, wz, d], -1)
        aux[:, :, 10 + d] = np.where(mask_z, 37 + piy[ii, wz, d], -1)
    aux[:, :, 14] = -mc * pdx_f[ii, wc]
    aux[:, :, 15] = -mc * pdy_f[ii, wc]
    aux[:, :, 16] = -mzf * cdx[ii, wz]
    aux[:, :, 17] = -mzf * cdy[ii, wz]

    # target-only constant (2 slots per group, d-independent)
    tc_c = mc * (1.0 + 1.0 / CLASSES
                 + 0.5 * (pdx_f[ii, wc] ** 2 + pdy_f[ii, wc] ** 2)
                 + 2.0 / GRID)
    tc_z = mzf * (1.0 + 1.0 / CLASSES
                  + 0.5 * (cdx[ii, wz] ** 2 + cdy[ii, wz] ** 2)
                  + 2.0 / GRID)
    c3 = 2.0 * (tc_c + tc_z).sum(axis=1)

    # [img, cell, f] -> [img, quarter, f, qcells]
    aux_q = aux.reshape(nb, 4, QCELLS, NAUX).transpose(0, 1, 3, 2)
    return np.ascontiguousarray(aux_q), c3.astype(np.float32)


def _wvec():
    import ml_dtypes

    w = np.empty(CH, np.float32)
    w[0] = 1.0
    w[1:1 + CLASSES] = 1.0 / CLASSES
    w[21:23] = 0.5
    w[23:] = 1.0 / GRID
    wv = np.zeros((NGRP, CHP), np.float32)
    wv[0, :CH] = w
    wv[1, :CH] = w
    wv[2:, :CH] = -2.0 * w
    return wv.reshape(-1).astype(ml_dtypes.bfloat16)


def kernel(out_four, bboxes, labels, H, W):
    global _PROG
    out_four = np.ascontiguousarray(np.asarray(out_four), dtype=np.float32)
    bboxes = np.asarray(bboxes, dtype=np.float32)
    labels = np.asarray(labels).astype(np.int64)

    aux_all, c3 = _prepare(bboxes, labels, float(np.asarray(W)))
    wv = _wvec()

    if _PROG is None:
        _PROG = _build_program()
    nc = _PROG

    in_maps = []
    for c in range(NCORES):
        sl = slice(c * IMGS_PER_CORE, (c + 1) * IMGS_PER_CORE)
        xs = out_four[sl].reshape(-1)
        a = aux_all[sl].reshape(NBATCH, IMGS_PER_BATCH * 4, NAUX * QCELLS)
        in_maps.append({"x": xs, "aux": np.ascontiguousarray(a), "wvec": wv})

    res = run_bass_kernel_spmd(nc, in_maps, list(range(NCORES)))

    out = np.empty((BATCH, 4), np.float32)
    for c in range(NCORES):
        acc = res.results[c]["acc"]          # [NBATCH, 128, NACC]
        acc = acc.reshape(NBATCH, IMGS_PER_BATCH, 4, 4, NCHUNK)
        dev = acc.sum(axis=(2, 4))           # [NBATCH, 32img, 4d]
        out[c * IMGS_PER_CORE:(c + 1) * IMGS_PER_CORE] = dev.reshape(
            IMGS_PER_CORE, 4)
    out += c3[:, None]
    return out


# revision 24
# speedup vs baseline: 1.1693x; 1.1693x over previous
"""PointLink loss kernel for 8 trn2 NeuronCores (data-parallel over batch).

loss[img, d] = sum_{cells, slots} W(cell,ch) * (o - T(cell,ch))^2
             = sum W*o^2  - 2*sum W*T*o  + sum W*T^2
A-pass: ACT square (fp16) + slot-pair sum + fused mult vs W tile
B-pass: slot-pair sum (Pool) + fused mult vs WT tile (WT = -2*W*T, built on
        device from per-cell indices via is_equal one-hots)
Reduce: one 4x tensor_scalar+accum over all products per (d, chunk)
C3: target-only constant, computed on host.

Device layout: partition p = (img_local 64, cell-half 2); per (d, chunk)
free dim = (cells 24/26, slot 4, ch 51), contiguous in DRAM. All 16-bit
tiles are fp16 with even-aligned runs so DVE 2x_1p/4x modes engage.
The host ships out_four already cast to fp16 (halves HBM traffic).
"""

import sys

sys.path.insert(0, "/opt/trn_rl_repo")

import numpy as np

import concourse.bass as bass
import concourse.bacc as bacc
import concourse.tile as tile
from concourse import mybir
from concourse.bass_utils import run_bass_kernel_spmd

GRID = 14
B = 2
CLASSES = 20
BATCH = 512
NBOX = 32
NCELL = GRID * GRID          # 196
CH = 51
SLOTS = 2 * B                # 4
SC = SLOTS * CH              # 204
PER_D = NCELL * SC           # 39984
PER_IMG = 4 * PER_D          # 159936
NCORES = 8
IMGS_PER_CORE = BATCH // NCORES   # 64
HCELLS = NCELL // 2               # 98 cells per partition (img, half)
CHUNKS = (24, 26, 24, 24)         # even cell chunks within a half
NCHUNK = len(CHUNKS)
NAUX = 18                         # per-cell aux fields
NGRP = 7                          # W_c, W_z, WT_c, WT_z d0..d3
GB = HCELLS * CH + 2              # 5000: T_all group block (cells 51-packed)
MAXC = max(CHUNKS)
SQB = MAXC * CH                   # 1326: per-slot block stride in sq tile
OSB = MAXC * CH + 2               # 1328: osum / prod block stride (even)
NACC = 4 * NCHUNK                 # one acc column per (d, chunk)

F32 = mybir.dt.float32
F16 = mybir.dt.float16


def _reap(ap, newap, extra_offset=0):
    return bass.AP(tensor=ap.tensor, offset=ap.offset + extra_offset, ap=newap)


# ----------------------------------------------------------------------------
# device program (built once per process)
# ----------------------------------------------------------------------------
_PROG = None


def _build_program():
    from contextlib import ExitStack

    nc = bacc.Bacc("TRN2", target_bir_lowering=False, debug=False,
                   num_devices=NCORES)
    x = nc.declare_dram_parameter("x", [IMGS_PER_CORE * PER_IMG], F16,
                                  isOutput=False)
    aux = nc.declare_dram_parameter("aux", [128, NAUX * HCELLS], F16,
                                    isOutput=False)
    wvec = nc.declare_dram_parameter("wvec", [2, 2 * CH], F16, isOutput=False)
    accout = nc.declare_dram_parameter("acc", [128, NACC], F32, isOutput=True)

    with tile.TileContext(nc) as tc, ExitStack() as ctx:
        singles = ctx.enter_context(tc.tile_pool(name="singles", bufs=1))
        opool = ctx.enter_context(tc.tile_pool(name="opool", bufs=3))
        sqpool = ctx.enter_context(tc.tile_pool(name="sqpool", bufs=2))
        ospool = ctx.enter_context(tc.tile_pool(name="ospool", bufs=2))
        prpool = ctx.enter_context(tc.tile_pool(name="prpool", bufs=2))
        accpool = ctx.enter_context(tc.tile_pool(name="accpool", bufs=1))

        # constants: row 0 = (w||w), row 1 = (-2w||-2w)
        t_wvec = singles.tile([128, 2, 2 * CH], F16)
        nc.gpsimd.dma_start(out=t_wvec[:], in_=wvec[:].partition_broadcast(128))
        t_iota = singles.tile([128, HCELLS, CH], F16)
        nc.gpsimd.iota(t_iota[:], pattern=[[0, HCELLS], [1, CH]], base=0,
                       channel_multiplier=0,
                       allow_small_or_imprecise_dtypes=True)

        # aux fields
        t_aux = singles.tile([128, NAUX, HCELLS], F16)
        nc.gpsimd.dma_start(out=t_aux[:], in_=aux[:])

        def fld_b(i, k):
            """aux field i broadcast over k columns: [128, HCELLS, k]"""
            a = t_aux[:, i, :]
            return _reap(a, [a.ap[0], [1, HCELLS], [0, k]])

        # resident target tile, fp16, cells 51-packed per group block
        t_all = singles.tile([128, NGRP * GB], F16)
        P = t_all[:].ap[0]

        def tview(grp, c0, ncols):
            return _reap(t_all[:], [P, [CH, HCELLS], [1, ncols]],
                         extra_offset=grp * GB + c0)

        def ioview(c0, ncols):
            a = t_iota[:]
            return _reap(a, [a.ap[0], [0, HCELLS], [1, ncols]],
                         extra_offset=c0)

        iseq = mybir.AluOpType.is_equal
        add = mybir.AluOpType.add
        mult = mybir.AluOpType.mult

        # ---- build targets ----
        # raw W_c / W_z: mask replicated over 51 ch
        nc.gpsimd.tensor_copy(out=tview(0, 0, CH), in_=fld_b(0, CH))
        nc.gpsimd.tensor_copy(out=tview(1, 0, CH), in_=fld_b(1, CH))
        # WT_c raw: conf=mask_c, cls+coord via is_equal (coord cols get 0),
        # links via is_equal
        nc.gpsimd.tensor_copy(out=tview(2, 0, 1), in_=fld_b(0, 1))
        nc.vector.tensor_tensor(out=tview(2, 1, 22), in0=fld_b(2, 22),
                                in1=ioview(1, 22), op=iseq)
        nc.vector.tensor_tensor(out=tview(2, 23, 14), in0=fld_b(3, 14),
                                in1=ioview(23, 14), op=iseq)
        nc.vector.tensor_tensor(out=tview(2, 37, 14), in0=fld_b(4, 14),
                                in1=ioview(37, 14), op=iseq)
        # WT_z d0 raw
        nc.gpsimd.tensor_copy(out=tview(3, 0, 1), in_=fld_b(1, 1))
        nc.vector.tensor_tensor(out=tview(3, 1, 22), in0=fld_b(5, 22),
                                in1=ioview(1, 22), op=iseq)
        # replicate z-common into d1..d3 before overwriting links
        for d in range(1, 4):
            nc.gpsimd.tensor_copy(out=tview(3 + d, 0, 23), in_=tview(3, 0, 23))
        # links per d
        for d in range(4):
            nc.vector.tensor_tensor(out=tview(3 + d, 23, 14),
                                    in0=fld_b(6 + d, 14),
                                    in1=ioview(23, 14), op=iseq)
            nc.vector.tensor_tensor(out=tview(3 + d, 37, 14),
                                    in0=fld_b(10 + d, 14),
                                    in1=ioview(37, 14), op=iseq)
        # scale each group by per-ch weights; cell-PAIR view keeps runs
        # 4B-aligned so fp16 2x engages
        for g in range(NGRP):
            wrow = 0 if g < 2 else 1
            wp = _reap(t_wvec[:],
                       [t_wvec[:].ap[0], [0, HCELLS // 2], [1, 2 * CH]],
                       extra_offset=wrow * 2 * CH)
            gv = _reap(t_all[:], [P, [2 * CH, HCELLS // 2], [1, 2 * CH]],
                       extra_offset=g * GB)
            nc.vector.tensor_tensor(out=gv, in0=gv, in1=wp, op=mult)
        # coord targets (host pre-scaled: -mask*crd), after scale
        crd_c = _reap(t_aux[:], [t_aux[:].ap[0], [1, HCELLS], [HCELLS, 2]],
                      extra_offset=14 * HCELLS)
        nc.gpsimd.tensor_copy(out=tview(2, 21, 2), in_=crd_c)
        crd_z = _reap(t_aux[:], [t_aux[:].ap[0], [1, HCELLS], [HCELLS, 2]],
                      extra_offset=16 * HCELLS)
        for d in range(4):
            nc.gpsimd.tensor_copy(out=tview(3 + d, 21, 2), in_=crd_z)
        # conf weight of W_c / W_z is 1 for every cell
        nc.vector.memset(tview(0, 0, 1), 1.0)
        nc.vector.memset(tview(1, 0, 1), 1.0)

        # ---- accumulators ----
        t_acc = accpool.tile([128, NACC], F32)

        # ---- main streaming loop ----
        for d in range(4):
            c0 = 0
            for ci, ncl in enumerate(CHUNKS):
                o_t = opool.tile([128, MAXC * SC], F16, tag="o")
                sq_t = sqpool.tile([128, SLOTS * SQB], F16, tag="sq")
                os_t = ospool.tile([128, 2 * OSB], F16, tag="os")
                pr_t = prpool.tile([128, 4 * OSB], F16, tag="prod")
                nfree = ncl * SC
                nf = ncl * CH
                src = _reap(
                    x[:],
                    [[PER_IMG, IMGS_PER_CORE], [HCELLS * SC, 2], [1, nfree]],
                    extra_offset=d * PER_D + c0 * SC)
                nc.sync.dma_start(out=o_t[:, 0:nfree], in_=src)

                op = o_t[:].ap[0]

                def oview(slot):
                    """o slot view [P, (cell), (ch)]"""
                    return _reap(o_t[:], [op, [SC, ncl], [1, CH]],
                                 extra_offset=slot * CH)

                def sqflat(slot):
                    return _reap(sq_t[:], [sq_t[:].ap[0], [1, nf]],
                                 extra_offset=slot * SQB)

                def osflat(grp):
                    return _reap(os_t[:], [os_t[:].ap[0], [1, nf]],
                                 extra_offset=grp * OSB)

                # squares per slot (fp16), packed contiguous per slot block
                for s in range(SLOTS):
                    so = _reap(sq_t[:], [sq_t[:].ap[0], [CH, ncl], [1, CH]],
                               extra_offset=s * SQB)
                    nc.scalar.activation(
                        out=so, in_=oview(s),
                        func=mybir.ActivationFunctionType.Square)
                # sq slot-pair sums in place (fp16 2x), both groups in one op
                sq2 = [sq_t[:].ap[0], [2 * SQB, 2], [1, nf]]
                nc.vector.tensor_tensor(
                    out=_reap(sq_t[:], sq2), in0=_reap(sq_t[:], sq2),
                    in1=_reap(sq_t[:], sq2, extra_offset=SQB), op=add)
                # o slot-pair sums on Pool (fp16)
                nc.gpsimd.tensor_tensor(out=osflat(0), in0=oview(0),
                                        in1=oview(1), op=add)
                nc.gpsimd.tensor_tensor(out=osflat(1), in0=oview(2),
                                        in1=oview(3), op=add)

                # merged weighted products (fp16 2x):
                # A: [sqsum_c | sqsum_z] * [W_c | W_z] -> prod segs 0,1
                pa = [pr_t[:].ap[0], [OSB, 2], [1, nf]]
                nc.vector.tensor_tensor(
                    out=_reap(pr_t[:], pa),
                    in0=_reap(sq_t[:], [sq_t[:].ap[0], [2 * SQB, 2], [1, nf]]),
                    in1=_reap(t_all[:], [P, [GB, 2], [1, nf]],
                              extra_offset=c0 * CH),
                    op=mult)
                # B: [osum_c | osum_z] * [WT_c | WT_z(d)] -> prod segs 2,3
                nc.vector.tensor_tensor(
                    out=_reap(pr_t[:], pa, extra_offset=2 * OSB),
                    in0=_reap(os_t[:], [os_t[:].ap[0], [OSB, 2], [1, nf]]),
                    in1=_reap(t_all[:], [P, [(1 + d) * GB, 2], [1, nf]],
                              extra_offset=2 * GB + c0 * CH),
                    op=mult)
                # one 4x reduce of all products into this (d, chunk) column
                flatpr = _reap(pr_t[:], [pr_t[:].ap[0], [OSB, 4], [1, nf]])
                col = d * NCHUNK + ci
                nc.vector.tensor_scalar(
                    out=flatpr, in0=flatpr, scalar1=1.0, scalar2=0.0,
                    op0=mult, op1=add,
                    accum_out=t_acc[:, col:col + 1])
                c0 += ncl

        nc.sync.dma_start(out=accout[:], in_=t_acc[:])

    nc.compile()
    return nc


# ----------------------------------------------------------------------------
# host-side target preparation
# ----------------------------------------------------------------------------
def _prepare(bboxes, labels, W):
    """Returns aux [BATCH, 2(half), NAUX, HCELLS] float16 and c3 [BATCH]."""
    g = bboxes.astype(np.float64) * (GRID / float(W))
    x0, y0, w, h = g[:, :, 0], g[:, :, 1], g[:, :, 2], g[:, :, 3]
    px = np.stack([x0, x0 + w, x0, x0 + w], axis=2)          # [B,N,4]
    py = np.stack([y0, y0, y0 + h, y0 + h], axis=2)
    px32 = px.astype(np.float32)
    py32 = py.astype(np.float32)
    pix = np.clip(np.floor(px32), 0, GRID - 1).astype(np.int64)
    piy = np.clip(np.floor(py32), 0, GRID - 1).astype(np.int64)
    pdx = (px32 - pix).astype(np.float32)
    pdy = (py32 - piy).astype(np.float32)
    cx = (x0 + w * 0.5).astype(np.float32)
    cy = (y0 + h * 0.5).astype(np.float32)
    cix = np.clip(np.floor(cx), 0, GRID - 1).astype(np.int64)
    ciy = np.clip(np.floor(cy), 0, GRID - 1).astype(np.int64)
    cdx = (cx - cix).astype(np.float32)
    cdy = (cy - ciy).astype(np.float32)

    nb = bboxes.shape[0]
    # corner first-match: flat scatter-min over (img, cell)
    corner_cell = (pix * GRID + piy).reshape(nb, 4 * NBOX)   # box-major
    which_c = np.full((nb, NCELL), 4 * NBOX, np.int64)
    imgix = np.repeat(np.arange(nb), 4 * NBOX)
    flat = imgix * NCELL + corner_cell.reshape(-1)
    wc_flat = which_c.reshape(-1)
    np.minimum.at(wc_flat, flat, np.tile(np.arange(4 * NBOX), nb))
    which_c = wc_flat.reshape(nb, NCELL)
    mask_c = which_c < 4 * NBOX
    wc = np.where(mask_c, which_c, 0)
    box_c = wc // 4

    center_cell = cix * GRID + ciy                            # [B,N]
    which_z = np.full((nb, NCELL), NBOX, np.int64)
    imgix2 = np.repeat(np.arange(nb), NBOX)
    flat2 = imgix2 * NCELL + center_cell.reshape(-1)
    wz_flat = which_z.reshape(-1)
    np.minimum.at(wz_flat, flat2, np.tile(np.arange(NBOX), nb))
    which_z = wz_flat.reshape(nb, NCELL)
    mask_z = which_z < NBOX
    wz = np.where(mask_z, which_z, 0)

    ii = np.arange(nb)[:, None]
    pdx_f = pdx.reshape(nb, -1)
    pdy_f = pdy.reshape(nb, -1)

    mc = mask_c.astype(np.float32)
    mzf = mask_z.astype(np.float32)
    aux = np.full((nb, NCELL, NAUX), -1.0, np.float32)
    aux[:, :, 0] = mc
    aux[:, :, 1] = mzf
    aux[:, :, 2] = np.where(mask_c, 1 + labels[ii, box_c], -1)
    aux[:, :, 3] = np.where(mask_c, 23 + cix[ii, box_c], -1)
    aux[:, :, 4] = np.where(mask_c, 37 + ciy[ii, box_c], -1)
    aux[:, :, 5] = np.where(mask_z, 1 + labels[ii, wz], -1)
    for d in range(4):
        aux[:, :, 6 + d] = np.where(mask_z, 23 + pix[ii, wz, d], -1)
        aux[:, :, 10 + d] = np.where(mask_z, 37 + piy[ii, wz, d], -1)
    aux[:, :, 14] = -mc * pdx_f[ii, wc]
    aux[:, :, 15] = -mc * pdy_f[ii, wc]
    aux[:, :, 16] = -mzf * cdx[ii, wz]
    aux[:, :, 17] = -mzf * cdy[ii, wz]

    # target-only constant (2 slots per group, d-independent).  Use the
    # fp16-rounded coord targets so C3 matches what the device multiplies.
    crd16 = aux[:, :, 14:18].astype(np.float16).astype(np.float64)
    tc_c = mc * (1.0 + 1.0 / CLASSES + 2.0 / GRID) \
        + 0.5 * (crd16[:, :, 0] ** 2 + crd16[:, :, 1] ** 2)
    tc_z = mzf * (1.0 + 1.0 / CLASSES + 2.0 / GRID) \
        + 0.5 * (crd16[:, :, 2] ** 2 + crd16[:, :, 3] ** 2)
    c3 = 2.0 * (tc_c + tc_z).sum(axis=1)

    # [img, cell, f] -> [img, half, f, hcells]
    aux_q = aux.reshape(nb, 2, HCELLS, NAUX).transpose(0, 1, 3, 2)
    return np.ascontiguousarray(aux_q).astype(np.float16), c3.astype(np.float32)


def _wvec():
    w = np.empty(CH, np.float32)
    w[0] = 1.0
    w[1:1 + CLASSES] = 1.0 / CLASSES
    w[21:23] = 0.5
    w[23:] = 1.0 / GRID
    ww = np.concatenate([w, w])
    return np.stack([ww, -2.0 * ww]).astype(np.float16)


def kernel(out_four, bboxes, labels, H, W):
    global _PROG
    out_four = np.asarray(out_four)
    bboxes = np.asarray(bboxes, dtype=np.float32)
    labels = np.asarray(labels).astype(np.int64)

    x16 = out_four.astype(np.float16)
    aux_all, c3 = _prepare(bboxes, labels, float(np.asarray(W)))
    wv = _wvec()

    if _PROG is None:
        _PROG = _build_program()
    nc = _PROG

    in_maps = []
    for c in range(NCORES):
        sl = slice(c * IMGS_PER_CORE, (c + 1) * IMGS_PER_CORE)
        xs = x16[sl].reshape(-1)
        a = aux_all[sl].reshape(IMGS_PER_CORE * 2, NAUX * HCELLS)
        in_maps.append({"x": xs, "aux": np.ascontiguousarray(a), "wvec": wv})

    res = run_bass_kernel_spmd(nc, in_maps, list(range(NCORES)))

    out = np.empty((BATCH, 4), np.float32)
    for c in range(NCORES):
        acc = res.results[c]["acc"]          # [128, NACC]
        acc = acc.reshape(IMGS_PER_CORE, 2, 4, NCHUNK)
        dev = acc.sum(axis=(1, 3))           # [64img, 4d]
        out[c * IMGS_PER_CORE:(c + 1) * IMGS_PER_CORE] = dev
    out += c3[:, None]
    return out
